# revision 28
# baseline (speedup 1.0000x reference)
"""CARAFE-downsampling Trainium2 kernel (8-core SPMD, full I/O contract).

Math (per core; batch n = core//4, output-row block s = core%4, h' in
[32s, 32s+32)):

  down+enc convs fused into 9 taps:  C_tap = B_tap @ A  (host, weights only)
      enc[e,hd,wd] = sum_tap C_tap @ xk[:, 2hd+dy, 2wd+dx]
      (xk = x rows [64s-1, 64s+64) + mask channel; mask gives exact conv
       zero-padding semantics through the fused 1x1)
  kw = softmax_e(enc)
  final 1x1 conv commuted before reassembly:
      G[co, r, u] = sum_{c,t} out_w[co, 4c+t] * x[c, 64t+16s-2+r, u-2]
      final[co, 2hd+q, w'] = out_b[co]
          + sum_{ki,kj} kw[5ki+kj, 2hd+q, w'] * G[co, hd+ki, 128q+w'+kj]

Layouts (v2, all tuned for DVE 2x mode: every elementwise operand is
bf16 with innermost stride 1 / count >= 2):
  G held transposed as t tiles [partition = u, free = (co, row)]; kj
  partition shifts via DMA replicas. kw tiles are tap-major with hd
  innermost: kwt[128, q, 25, 16]. Per (q, oct) block products multiply
  t[:, :, 8oct+ki:...+8] by kw broadcast over co; DVE taps reduce via a
  vectorized halving tree, Pool taps go stg -> PE identity-matmul PSUM
  accumulation (seeded with the output bias). Final combine on PE (tree
  root added into PSUM), result copied out by the scalar engine.
"""
import os

import numpy as np
import ml_dtypes

import concourse.bass as bass
import concourse.tile as tile
from concourse import bacc, mybir, masks
from concourse.bass_utils import run_bass_kernel_spmd

F32 = mybir.dt.float32
BF16 = mybir.dt.bfloat16

N_CORES = 8
K5 = 5

# ---- product routing (tunable) ----------------------------------------------
# Products are emitted as ki-grouped ops (one op covers taps 5*ki+kj for a
# run of ki at fixed kj, reading an overlapping row window of the G tile).
# Tree slots 0-14 hold kj0 (0-4), kj1 (5-9), kj2-ki0-2 (10-12, DVE),
# kj2-ki3-4 (13-14, Pool); kj3/kj4 go to stg -> PE identity accumulation.
TREE_GROUPS_V = [(0, 0, 5, 0), (1, 0, 5, 5), (2, 0, 3, 10)]  # kj, ki0, n, slot
TREE_GROUPS_P = []
STG_GROUPS_P = [(2, 3, 2, 0), (3, 0, 5, 2), (4, 0, 5, 7)]
N_TREE = 13
N_STG = 12
# tree levels split across DVE ('V') and Pool ('P'):
# (dst_lo, dst_hi, src_lo, src_hi, eng)
TREE_OPS = [
    (0, 4, 7, 11, 'V'), (4, 6, 11, 13, 'P'),
    (0, 2, 4, 6, 'V'), (2, 3, 6, 7, 'P'),
    (0, 2, 2, 4, 'V'),
    (0, 1, 1, 2, 'V'),
]


# ----------------------------------------------------------------------------
# device program
# ----------------------------------------------------------------------------
def build_nc():
    nc = bacc.Bacc(None, target_bir_lowering=False)

    xk_d = nc.dram_tensor("xk", [65, 65, 258], BF16, kind="ExternalInput")
    xb_d = nc.dram_tensor("xb", [2, 128, 20, 264], BF16, kind="ExternalInput")
    ct_d = nc.dram_tensor("ctap", [65, 9, 25], BF16, kind="ExternalInput")
    w4_d = nc.dram_tensor("w4", [2, 128, 64], BF16, kind="ExternalInput")
    ob_d = nc.dram_tensor("obf", [128, 64, 8], BF16, kind="ExternalInput")
    # out[q, oct, w', co, hh] -> host transposes to (co, h', w')
    out_d = nc.dram_tensor("out", [2, 2, 128, 64, 8], BF16, kind="ExternalOutput")

    ctx = nc.allow_low_precision(reason="bf16 pipeline; validated ~1% rel err")
    ctx.__enter__()
    with tile.TileContext(nc) as tc:
        with (
            tc.tile_pool(name="consts", bufs=1) as consts,
            tc.tile_pool(name="xkp", bufs=4) as xkp,
            tc.tile_pool(name="xbp", bufs=1) as xbp,
            tc.tile_pool(name="encp", bufs=1) as encp,
            tc.tile_pool(name="kwp", bufs=1) as kwp,
            tc.tile_pool(name="tbp", bufs=1) as tbp,
            tc.tile_pool(name="prodp", bufs=3) as prodp,
            tc.tile_pool(name="stgp", bufs=3) as stgp,
            tc.tile_pool(name="resp", bufs=3) as resp,
            tc.tile_pool(name="pse", bufs=1, space="PSUM") as pse,
            tc.tile_pool(name="psg", bufs=2, space="PSUM") as psg,
            tc.tile_pool(name="pst", bufs=1, space="PSUM") as pst,
            tc.tile_pool(name="psacc", bufs=2, space="PSUM") as psacc,
        ):
            # ---- inputs (SP: ctap, xk, w4/obf; Act: xk0-half, xb) ----
            ctap = consts.tile([65, 9, 25], BF16)
            nc.sync.dma_start(ctap[:], ct_d[:])
            identb = consts.tile([128, 128], BF16)
            masks.make_identity(nc, identb[:])
            ident = consts.tile([32, 32], F32)
            masks.make_identity(nc, ident[:])

            # xk in 4 chunks of 18 rows (advance 16); chunk c rows [16c,16c+18)
            # chunk 0 split across SP+Act so enc can start ~2us earlier.
            xkts = [xkp.tile([65, 18, 258], BF16, tag="xk", name=f"xkt{c}")
                    for c in range(4)]
            nc.sync.dma_start(xkts[0][:, 0:9, :], xk_d[:, 0:9, :])
            nc.scalar.dma_start(xkts[0][:, 9:18, :], xk_d[:, 9:18, :])
            nc.sync.dma_start(xkts[1][:], xk_d[:, 16:34, :])
            w4 = [consts.tile([128, 64], BF16, name=f"w4_{i}", tag=f"w4_{i}")
                  for i in range(2)]
            for i in range(2):
                nc.sync.dma_start(w4[i][:], w4_d[i])
            obf16 = consts.tile([128, 64, 8], BF16)
            nc.sync.dma_start(obf16[:], ob_d[:])
            nc.sync.dma_start(xkts[2][:], xk_d[:, 32:50, :])
            nc.sync.dma_start(xkts[3][:, 0:17, :], xk_d[:, 48:65, :])

            xb = [xbp.tile([128, 20, 264], BF16, name=f"xb{i}", tag=f"xb{i}")
                  for i in range(2)]
            for i in range(2):
                nc.scalar.dma_start(xb[i][:], xb_d[i])

            # per-half tiles so oct-0 consumers don't wait on the oct-1 half
            enc_sb = [encp.tile([25, 16, 128], F32, name=f"enc{h}",
                                tag=f"enc{h}") for h in range(2)]
            # kw tap-major, hd innermost: [w', q, tap, hh]
            kwt = [kwp.tile([128, 2, 25, 8], BF16, name=f"kwt{h}",
                            tag=f"kwt{h}") for h in range(2)]
            zrec = [kwp.tile([128, 8], BF16, name=f"zrec{h}",
                             tag=f"zrec{h}") for h in range(2)]

            # ---- enc: sub-chunk j covers h' rows [4j, 4j+4) ----
            def enc_sub(j):
                xkt = xkts[j // 2]
                ro = 8 * (j % 2)
                pe = pse.tile([25, 4, 128], F32, name=f"pe{j}", tag=f"pe{j % 2}")
                first = True
                for dy in range(3):
                    for dx in range(3):
                        rhs = xkt[:, ro + dy:ro + dy + 8:2, dx:dx + 256:2]
                        nc.tensor.matmul(
                            pe[:], ctap[:, 3 * dy + dx, :], rhs,
                            start=first, stop=(dy == 2 and dx == 2),
                        )
                        first = False
                # half-0 drains on DVE (idle that early); half-1 on Act
                # (GPSIMD can't touch PSUM on HW)
                dst = enc_sb[j // 4][:, 4 * (j % 4):4 * (j % 4) + 4, :]
                if j < 4:
                    nc.vector.tensor_copy(dst, pe[:])
                else:
                    nc.scalar.copy(dst, pe[:])

            # ---- G, u-major: t tiles [u, co, row] ----
            # tb[0]: u in [0,128), tb[1]: [128,256), tbt: [256,264)
            tb = [tbp.tile([128, 64, 20], BF16, name=f"tb{c}", tag=f"tb{c}")
                  for c in range(2)]
            tbt = tbp.tile([8, 64, 20], BF16)
            UCH = [(0, 128, tb[0]), (128, 128, tb[1]), (256, 8, tbt)]

            def g_rowgroup(rg, uc):
                u0, w, dst = UCH[uc]
                pg = psg.tile([128, 4, 64], F32, tag="pg", name=f"pg{rg}_{uc}")
                for rr in range(4):
                    r = 4 * rg + rr
                    for ci in range(2):
                        nc.tensor.matmul(
                            pg[0:w, rr, :], xb[ci][:, r, u0:u0 + w], w4[ci][:],
                            start=(ci == 0), stop=(ci == 1),
                        )
                # permuted copy (row, co) -> (co, row); GPSIMD cannot read
                # PSUM on real HW, so this rides the Act queue
                nc.scalar.copy(
                    dst[0:w, :, 4 * rg:4 * rg + 4],
                    pg[0:w].transpose([0, 2, 1]),
                )

            def kw_mm_exp(half):
                # PE transposes + Act exp for h' rows [16half, 16half+16)
                pts = []
                for q in range(2):
                    pt = pst.tile([128, 8, 25], F32, name=f"pt{q}{half}",
                                  tag=f"pt{q}")
                    for j in range(8):
                        nc.tensor.matmul(
                            pt[:, j, :], enc_sb[half][:, 2 * j + q, :],
                            ident[:25, :25], is_transpose=True,
                        )
                    pts.append(pt)
                for q in range(2):
                    # exp: write transposed into tap-major kwt
                    nc.scalar.activation(
                        kwt[half][:, q].transpose([0, 2, 1]),
                        pts[q][:], mybir.ActivationFunctionType.Exp,
                    )

            def kw_norm(half):
                for q in range(2):
                    zsum = resp.tile([128, 8], F32, tag=f"zs{q}",
                                     name=f"zs{q}{half}")
                    nc.vector.tensor_reduce(
                        zsum[:], kwt[half][:, q].transpose([0, 2, 1]),
                        axis=mybir.AxisListType.X, op=mybir.AluOpType.add,
                    )
                    nc.vector.reciprocal(zrec[half][:], zsum[:])
                    nc.vector.tensor_mul(
                        kwt[half][:, q], kwt[half][:, q],
                        zrec[half][:].unsqueeze(1).broadcast_to([128, 25, 8]),
                    )

            # ---- phase 1: enc 0-3 -> kw half 0; G (u-block 0 first) ----
            enc_sub(0)
            enc_sub(1)
            enc_sub(2)
            enc_sub(3)
            for rg in range(5):
                g_rowgroup(rg, 0)
            kw_mm_exp(0)
            kw_norm(0)
            for uc in (1, 2):
                for rg in range(5):
                    g_rowgroup(rg, uc)

            # ---- replicas: trep[(q, kj)][w'] = G col 128q + w' + kj ----
            # SP (free after xk) takes most, ordered by when products need
            # them; Pool takes the two feeding its own first stg products.
            trep = {}
            for q in range(2):
                trep[(q, 0)] = tb[q]
            repl_engines = {
                (0, 1): nc.sync, (1, 1): nc.sync,
                (0, 2): nc.sync, (1, 2): nc.sync,
                (0, 3): nc.gpsimd, (0, 4): nc.gpsimd,
                (1, 3): nc.sync, (1, 4): nc.sync,
            }
            repl_order = [(0, 1), (0, 3), (1, 1), (0, 4), (0, 2), (1, 2),
                          (1, 3), (1, 4)]
            for q, kj in repl_order:
                t = tbp.tile([128, 64, 20], BF16, name=f"tr{q}{kj}",
                             tag=f"tr{q}{kj}")
                eng = repl_engines[(q, kj)]
                eng.dma_start(t[0:128 - kj], tb[q][kj:128])
                srct = tb[1][0:kj] if q == 0 else tbt[0:kj]
                eng.dma_start(t[128 - kj:128], srct)
                trep[(q, kj)] = t

            # ---- phase 2: enc 4-7 (PSUM drains on Act), kwT1 mm ----
            enc_sub(4)
            enc_sub(5)
            enc_sub(6)
            enc_sub(7)
            kw_mm_exp(1)

            # ---- products + reduction per (q, oct) block ----
            def prod_block(q, oct_):
                prod = prodp.tile([128, N_TREE, 64, 8], BF16, tag="prod",
                                  name=f"prod{q}{oct_}")
                stg = stgp.tile([128, N_STG, 64, 8], BF16, tag="stg",
                                name=f"stg{q}{oct_}")
                acc = psacc.tile([128, 64, 8], F32, tag="acc",
                                 name=f"acc{q}{oct_}")
                # seed with output bias
                nc.tensor.matmul(acc[:], identb[:], obf16[:],
                                 start=True, stop=False)

                def tview(kj, ki0, nki):
                    # overlapping window: (ki, co, hh) over G rows
                    # 8*oct_ + ki0 + ki + hh
                    base = trep[(q, kj)][:]
                    return bass.AP(
                        base.tensor, base.offset + 8 * oct_ + ki0,
                        [list(base.ap[0]), [1, nki], [20, 64], [1, 8]])

                def kw_gview(kj, ki0, nki):
                    return (kwt[oct_][:, q, 5 * ki0 + kj::5, :][:, 0:nki, :]
                            .unsqueeze(2).broadcast_to([128, nki, 64, 8]))

                for kj, ki0, nki, s0 in TREE_GROUPS_V:
                    nc.vector.tensor_mul(prod[:, s0:s0 + nki],
                                         tview(kj, ki0, nki),
                                         kw_gview(kj, ki0, nki))
                # Pool: its tree-slot group, then its tree levels (before the
                # stg products so DVE's dependent levels aren't stuck)
                for kj, ki0, nki, s0 in TREE_GROUPS_P:
                    nc.gpsimd.tensor_mul(prod[:, s0:s0 + nki],
                                         tview(kj, ki0, nki),
                                         kw_gview(kj, ki0, nki))
                for lo, hi, slo, shi, eng_c in TREE_OPS:
                    if eng_c == 'P':
                        nc.gpsimd.tensor_add(prod[:, lo:hi], prod[:, lo:hi],
                                             prod[:, slo:shi])
                for kj, ki0, nki, s0 in STG_GROUPS_P:
                    nc.gpsimd.tensor_mul(stg[:, s0:s0 + nki],
                                         tview(kj, ki0, nki),
                                         kw_gview(kj, ki0, nki))
                    for si in range(s0, s0 + nki):
                        nc.tensor.matmul(acc[:], identb[:], stg[:, si, :, :],
                                         start=False, stop=False)
                for lo, hi, slo, shi, eng_c in TREE_OPS:
                    if eng_c == 'V':
                        nc.vector.tensor_add(prod[:, lo:hi], prod[:, lo:hi],
                                             prod[:, slo:shi])
                # add tree root into PSUM acc on PE; copy out on scalar
                nc.tensor.matmul(acc[:], identb[:], prod[:, 0, :, :],
                                 start=False, stop=True)
                res = resp.tile([128, 64, 8], BF16, tag="res",
                                name=f"res{q}{oct_}")
                nc.scalar.copy(res[:], acc[:])
                nc.sync.dma_start(out_d[q, oct_], res[:])

            prod_block(0, 0)
            prod_block(1, 0)
            kw_norm(1)
            prod_block(0, 1)
            prod_block(1, 1)

    nc.compile()
    ctx.__exit__(None, None, None)
    return nc


# ----------------------------------------------------------------------------
# host side
# ----------------------------------------------------------------------------
def _prep_weights(down_w, down_b, enc_w, enc_b, out_w, out_b):
    A = np.zeros((65, 65), np.float32)
    A[0:64, 0:64] = down_w[:, :, 0, 0]
    A[0:64, 64] = down_b
    A[64, 64] = 1.0
    ctap = np.zeros((65, 9, 25), np.float32)
    for dy in range(3):
        for dx in range(3):
            B = np.zeros((25, 65), np.float32)
            B[:, 0:64] = enc_w[:, :, dy, dx]
            if dy == 1 and dx == 1:
                B[:, 64] = enc_b
            ctap[:, 3 * dy + dx, :] = (B @ A).T
    w4 = out_w[:, :, 0, 0].T.reshape(2, 128, 64).astype(ml_dtypes.bfloat16)
    obf = np.broadcast_to(out_b[None, :, None], (128, 64, 8)).astype(
        ml_dtypes.bfloat16)
    return ctap.astype(ml_dtypes.bfloat16), w4, obf


def _slice_core(x, n, s):
    xk = np.zeros((65, 65, 258), np.float32)
    h0 = 64 * s - 1
    lo, hi = max(0, -h0), min(65, 256 - h0)
    xk[0:64, lo:hi, 1:257] = x[n, :, h0 + lo:h0 + hi, :]
    xk[64, lo:hi, 1:257] = 1.0
    xb = np.zeros((2, 128, 20, 264), np.float32)
    xbv = xb.reshape(256, 20, 264)
    for t in range(4):
        g0 = 64 * t + 16 * s - 2
        lo, hi = max(0, -g0), min(20, 256 - g0)
        xbv[np.arange(64) * 4 + t, lo:hi, 2:258] = x[n, :, g0 + lo:g0 + hi, :]
    return xk.astype(ml_dtypes.bfloat16), xb.astype(ml_dtypes.bfloat16)


_NC_CACHE = None
LAST_EXEC_NS = None


def kernel(x, down_w, down_b, enc_w, enc_b, out_w, out_b):
    global _NC_CACHE, LAST_EXEC_NS
    x = np.asarray(x, np.float32)
    ctap, w4, obf = _prep_weights(
        np.asarray(down_w, np.float32), np.asarray(down_b, np.float32),
        np.asarray(enc_w, np.float32), np.asarray(enc_b, np.float32),
        np.asarray(out_w, np.float32), np.asarray(out_b, np.float32))
    in_maps = []
    for core in range(N_CORES):
        n, s = core // 4, core % 4
        xk, xb = _slice_core(x, n, s)
        in_maps.append({"xk": xk, "xb": xb, "ctap": ctap, "w4": w4, "obf": obf})
    if _NC_CACHE is None:
        _NC_CACHE = build_nc()
    kw = {}
    if os.environ.get("CARAFE_TRACE"):
        kw = dict(trace=True, tmpdir=os.environ.get("CARAFE_TRACE_DIR"))
    res = run_bass_kernel_spmd(_NC_CACHE, in_maps, list(range(N_CORES)), **kw)
    if res.exec_time_ns is not None:
        LAST_EXEC_NS = res.exec_time_ns
    out = np.zeros((2, 64, 128, 128), np.float32)
    for core in range(N_CORES):
        n, s = core // 4, core % 4
        o = res.results[core]["out"].astype(np.float32)  # (q, oct, w', co, hh)
        # h' = 16*oct + 2*hh + q  ->  (co, h', w')
        o = o.transpose(3, 1, 4, 0, 2).reshape(64, 32, 128)
        out[n, :, 32 * s:32 * s + 32, :] = o
    return out


# revision 33
# speedup vs baseline: 1.2358x; 1.2358x over previous
"""CARAFE-downsampling Trainium2 kernel (8-core SPMD, full I/O contract).

Math (per core; batch n = core//4, output-row block s = core%4, h' in
[32s, 32s+32)):

  down+enc convs fused into 9 taps:  C_tap = B_tap @ A  (host, weights only)
      enc[e,hd,wd] = sum_tap C_tap @ xk[:, 2hd+dy, 2wd+dx]
      (xk = x rows [64s-1, 64s+64) + mask channel; mask gives exact conv
       zero-padding semantics through the fused 1x1)
  kw = softmax_e(enc)
  final 1x1 conv commuted before reassembly:
      G[co, r, u] = sum_{c,t} out_w[co, 4c+t] * x[c, 64t+16s-2+r, u-2]
      final[co, 2hd+q, w'] = out_b[co]
          + sum_{ki,kj} kw[5ki+kj, 2hd+q, w'] * G[co, hd+ki, 128q+w'+kj]

Layouts (v2, all tuned for DVE 2x mode: every elementwise operand is
bf16 with innermost stride 1 / count >= 2):
  G held transposed as t tiles [partition = u, free = (co, row)]; kj
  partition shifts via DMA replicas. kw tiles are tap-major with hd
  innermost: kwt[128, q, 25, 16]. Per (q, oct) block products multiply
  t[:, :, 8oct+ki:...+8] by kw broadcast over co; DVE taps reduce via a
  vectorized halving tree, Pool taps go stg -> PE identity-matmul PSUM
  accumulation (seeded with the output bias). Final combine on PE (tree
  root added into PSUM), result copied out by the scalar engine.
"""
import os

import numpy as np
import ml_dtypes

import concourse.bass as bass
import concourse.tile as tile
from concourse import bacc, mybir, masks
from concourse.bass_utils import run_bass_kernel_spmd

F32 = mybir.dt.float32
BF16 = mybir.dt.bfloat16

N_CORES = 8
K5 = 5

# ---- product routing (tunable) ----------------------------------------------
# Products are emitted as ki-grouped ops (one op covers taps 5*ki+kj for a
# run of ki at fixed kj, reading an overlapping row window of the G tile).
# Tree slots 0-14 hold kj0 (0-4), kj1 (5-9), kj2-ki0-2 (10-12, DVE),
# kj2-ki3-4 (13-14, Pool); kj3/kj4 go to stg -> PE identity accumulation.
TREE_GROUPS_V = [(0, 0, 5, 0), (1, 0, 5, 5), (2, 0, 3, 10)]  # kj, ki0, n, slot
TREE_GROUPS_P = []
STG_GROUPS_P = [(2, 3, 2, 0), (3, 0, 5, 2), (4, 0, 5, 7)]
N_TREE = 13
N_STG = 12
# tree levels split across DVE ('V') and Pool ('P'):
# (dst_lo, dst_hi, src_lo, src_hi, eng)
TREE_OPS = [
    (0, 4, 7, 11, 'V'), (4, 6, 11, 13, 'P'),
    (0, 2, 4, 6, 'V'), (2, 3, 6, 7, 'V'),
    (0, 2, 2, 4, 'V'),
    (0, 1, 1, 2, 'V'),
]


# ----------------------------------------------------------------------------
# device program
# ----------------------------------------------------------------------------
def build_nc():
    nc = bacc.Bacc(None, target_bir_lowering=False)

    xk_d = nc.dram_tensor("xk", [65, 65, 258], BF16, kind="ExternalInput")
    xb_d = nc.dram_tensor("xb", [2, 128, 20, 264], BF16, kind="ExternalInput")
    ct_d = nc.dram_tensor("ctap", [65, 9, 25], BF16, kind="ExternalInput")
    w4_d = nc.dram_tensor("w4", [2, 128, 64], BF16, kind="ExternalInput")
    ob_d = nc.dram_tensor("obf", [128, 64, 8], BF16, kind="ExternalInput")
    # out[q, oct, w', co, hh] -> host transposes to (co, h', w')
    out_d = nc.dram_tensor("out", [2, 2, 128, 64, 8], BF16, kind="ExternalOutput")

    ctx = nc.allow_low_precision(reason="bf16 pipeline; validated ~1% rel err")
    ctx.__enter__()
    with tile.TileContext(nc) as tc:
        with (
            tc.tile_pool(name="consts", bufs=1) as consts,
            tc.tile_pool(name="xkp", bufs=4) as xkp,
            tc.tile_pool(name="xbp", bufs=1) as xbp,
            tc.tile_pool(name="encp", bufs=1) as encp,
            tc.tile_pool(name="kwp", bufs=1) as kwp,
            tc.tile_pool(name="tbp", bufs=1) as tbp,
            tc.tile_pool(name="prodp", bufs=3) as prodp,
            tc.tile_pool(name="stgp", bufs=3) as stgp,
            tc.tile_pool(name="resp", bufs=2) as resp,
            tc.tile_pool(name="pse", bufs=1, space="PSUM") as pse,
            tc.tile_pool(name="psg", bufs=2, space="PSUM") as psg,
            tc.tile_pool(name="pst", bufs=1, space="PSUM") as pst,
            tc.tile_pool(name="psacc", bufs=2, space="PSUM") as psacc,
        ):
            # ---- inputs (SP: ctap, xk, w4/obf; Act: xk0-half, xb) ----
            ctap = consts.tile([65, 9, 25], BF16)
            nc.sync.dma_start(ctap[:], ct_d[:])
            identb = consts.tile([128, 128], BF16)
            masks.make_identity(nc, identb[:])
            ident = consts.tile([32, 32], F32)
            masks.make_identity(nc, ident[:])

            # xk in 4 chunks of 18 rows (advance 16); chunk c rows [16c,16c+18)
            # chunk 0 split across SP+Act so enc can start ~2us earlier.
            xkts = [xkp.tile([65, 18, 258], BF16, tag="xk", name=f"xkt{c}")
                    for c in range(4)]
            nc.sync.dma_start(xkts[0][:, 0:9, :], xk_d[:, 0:9, :])
            nc.scalar.dma_start(xkts[0][:, 9:18, :], xk_d[:, 9:18, :])
            nc.sync.dma_start(xkts[1][:], xk_d[:, 16:34, :])
            w4 = [consts.tile([128, 64], BF16, name=f"w4_{i}", tag=f"w4_{i}")
                  for i in range(2)]
            for i in range(2):
                nc.sync.dma_start(w4[i][:], w4_d[i])
            obf16 = consts.tile([128, 64, 8], BF16)
            nc.sync.dma_start(obf16[:], ob_d[:])
            nc.sync.dma_start(xkts[2][:], xk_d[:, 32:50, :])
            nc.sync.dma_start(xkts[3][:, 0:17, :], xk_d[:, 48:65, :])

            xb = [xbp.tile([128, 20, 264], BF16, name=f"xb{i}", tag=f"xb{i}")
                  for i in range(2)]
            for i in range(2):
                nc.scalar.dma_start(xb[i][:], xb_d[i])

            # per-half tiles so oct-0 consumers don't wait on the oct-1 half
            enc_sb = [encp.tile([25, 16, 128], F32, name=f"enc{h}",
                                tag=f"enc{h}") for h in range(2)]
            # kw tap-major, hd innermost: [w', q, tap, hh]
            kwt = [kwp.tile([128, 2, 25, 8], BF16, name=f"kwt{h}",
                            tag=f"kwt{h}") for h in range(2)]
            zrec = [kwp.tile([128, 8], BF16, name=f"zrec{h}",
                             tag=f"zrec{h}") for h in range(2)]

            # ---- enc: sub-chunk j covers h' rows [4j, 4j+4) ----
            def enc_sub(j):
                xkt = xkts[j // 2]
                ro = 8 * (j % 2)
                pe = pse.tile([25, 4, 128], F32, name=f"pe{j}", tag=f"pe{j % 2}")
                first = True
                for dy in range(3):
                    for dx in range(3):
                        rhs = xkt[:, ro + dy:ro + dy + 8:2, dx:dx + 256:2]
                        nc.tensor.matmul(
                            pe[:], ctap[:, 3 * dy + dx, :], rhs,
                            start=first, stop=(dy == 2 and dx == 2),
                        )
                        first = False
                # half-0 drains on DVE (idle that early); half-1 on Act
                # (GPSIMD can't touch PSUM on HW)
                dst = enc_sb[j // 4][:, 4 * (j % 4):4 * (j % 4) + 4, :]
                if j < 4:
                    nc.vector.tensor_copy(dst, pe[:])
                else:
                    nc.scalar.copy(dst, pe[:])

            # ---- G, u-major: t tiles [u, co, row] ----
            # tb[0]: u in [0,128), tb[1]: [128,256), tbt: [256,264)
            tb = [tbp.tile([128, 64, 20], BF16, name=f"tb{c}", tag=f"tb{c}")
                  for c in range(2)]
            tbt = tbp.tile([8, 64, 20], BF16)
            UCH = [(0, 128, tb[0]), (128, 128, tb[1]), (256, 8, tbt)]

            def g_rowgroup(rg, uc):
                u0, w, dst = UCH[uc]
                pg = psg.tile([128, 4, 64], F32, tag="pg", name=f"pg{rg}_{uc}")
                for rr in range(4):
                    r = 4 * rg + rr
                    for ci in range(2):
                        nc.tensor.matmul(
                            pg[0:w, rr, :], xb[ci][:, r, u0:u0 + w], w4[ci][:],
                            start=(ci == 0), stop=(ci == 1),
                        )
                # permuted copy (row, co) -> (co, row); GPSIMD cannot read
                # PSUM on real HW, so this rides the Act queue
                nc.scalar.copy(
                    dst[0:w, :, 4 * rg:4 * rg + 4],
                    pg[0:w].transpose([0, 2, 1]),
                )

            def kw_mm_exp(half):
                # PE transposes + Act exp for h' rows [16half, 16half+16)
                pts = []
                for q in range(2):
                    pt = pst.tile([128, 8, 25], F32, name=f"pt{q}{half}",
                                  tag=f"pt{q}")
                    for j in range(8):
                        nc.tensor.matmul(
                            pt[:, j, :], enc_sb[half][:, 2 * j + q, :],
                            ident[:25, :25], is_transpose=True,
                        )
                    pts.append(pt)
                for q in range(2):
                    # exp: write transposed into tap-major kwt
                    nc.scalar.activation(
                        kwt[half][:, q].transpose([0, 2, 1]),
                        pts[q][:], mybir.ActivationFunctionType.Exp,
                    )

            def kw_norm(half):
                for q in range(2):
                    zsum = resp.tile([128, 8], F32, tag=f"zs{q}",
                                     name=f"zs{q}{half}")
                    nc.vector.tensor_reduce(
                        zsum[:], kwt[half][:, q].transpose([0, 2, 1]),
                        axis=mybir.AxisListType.X, op=mybir.AluOpType.add,
                    )
                    nc.vector.reciprocal(zrec[half][:], zsum[:])
                    nc.vector.tensor_mul(
                        kwt[half][:, q], kwt[half][:, q],
                        zrec[half][:].unsqueeze(1).broadcast_to([128, 25, 8]),
                    )

            # ---- phase 1: enc 0-3 -> kw half 0; G (u-block 0 first) ----
            enc_sub(0)
            enc_sub(1)
            enc_sub(2)
            enc_sub(3)
            for rg in range(5):
                g_rowgroup(rg, 0)
            kw_mm_exp(0)
            kw_norm(0)
            for uc in (1, 2):
                for rg in range(5):
                    g_rowgroup(rg, uc)

            # ---- replicas: trep[(q, kj)][w'] = G col 128q + w' + kj ----
            # SP (free after xk) takes most, ordered by when products need
            # them; Pool takes the two feeding its own first stg products.
            trep = {}
            for q in range(2):
                trep[(q, 0)] = tb[q]
            repl_engines = {
                (0, 1): nc.sync, (1, 1): nc.sync,
                (0, 2): nc.sync, (1, 2): nc.sync,
                (0, 3): nc.gpsimd, (0, 4): nc.gpsimd,
                (1, 3): nc.sync, (1, 4): nc.sync,
            }
            repl_order = [(0, 1), (0, 3), (1, 1), (0, 4), (0, 2), (1, 2),
                          (1, 3), (1, 4)]
            for q, kj in repl_order:
                t = tbp.tile([128, 64, 20], BF16, name=f"tr{q}{kj}",
                             tag=f"tr{q}{kj}")
                eng = repl_engines[(q, kj)]
                eng.dma_start(t[0:128 - kj], tb[q][kj:128])
                srct = tb[1][0:kj] if q == 0 else tbt[0:kj]
                eng.dma_start(t[128 - kj:128], srct)
                trep[(q, kj)] = t

            # ---- phase 2: enc 4-7 (PSUM drains on Act), kwT1 mm ----
            enc_sub(4)
            enc_sub(5)
            enc_sub(6)
            enc_sub(7)
            kw_mm_exp(1)

            # ---- products + reduction per (q, oct) block ----
            def prod_block(q, oct_):
                prod = prodp.tile([128, N_TREE, 64, 8], BF16, tag="prod",
                                  name=f"prod{q}{oct_}")
                stg = stgp.tile([128, N_STG, 64, 8], BF16, tag="stg",
                                name=f"stg{q}{oct_}")
                acc = psacc.tile([128, 64, 8], F32, tag="acc",
                                 name=f"acc{q}{oct_}")
                # seed with output bias
                nc.tensor.matmul(acc[:], identb[:], obf16[:],
                                 start=True, stop=False)

                def tview(kj, ki0, nki):
                    # overlapping window: (ki, co, hh) over G rows
                    # 8*oct_ + ki0 + ki + hh
                    base = trep[(q, kj)][:]
                    return bass.AP(
                        base.tensor, base.offset + 8 * oct_ + ki0,
                        [list(base.ap[0]), [1, nki], [20, 64], [1, 8]])

                def kw_gview(kj, ki0, nki):
                    return (kwt[oct_][:, q, 5 * ki0 + kj::5, :][:, 0:nki, :]
                            .unsqueeze(2).broadcast_to([128, nki, 64, 8]))

                for kj, ki0, nki, s0 in TREE_GROUPS_V:
                    nc.vector.tensor_mul(prod[:, s0:s0 + nki],
                                         tview(kj, ki0, nki),
                                         kw_gview(kj, ki0, nki))
                # Pool: its tree-slot group, then its tree levels (before the
                # stg products so DVE's dependent levels aren't stuck)
                for kj, ki0, nki, s0 in TREE_GROUPS_P:
                    nc.gpsimd.tensor_mul(prod[:, s0:s0 + nki],
                                         tview(kj, ki0, nki),
                                         kw_gview(kj, ki0, nki))
                for lo, hi, slo, shi, eng_c in TREE_OPS:
                    if eng_c == 'P':
                        nc.gpsimd.tensor_add(prod[:, lo:hi], prod[:, lo:hi],
                                             prod[:, slo:shi])
                for kj, ki0, nki, s0 in STG_GROUPS_P:
                    nc.gpsimd.tensor_mul(stg[:, s0:s0 + nki],
                                         tview(kj, ki0, nki),
                                         kw_gview(kj, ki0, nki))
                    for si in range(s0, s0 + nki):
                        nc.tensor.matmul(acc[:], identb[:], stg[:, si, :, :],
                                         start=False, stop=False)
                for lo, hi, slo, shi, eng_c in TREE_OPS:
                    if eng_c == 'V':
                        nc.vector.tensor_add(prod[:, lo:hi], prod[:, lo:hi],
                                             prod[:, slo:shi])
                # add tree root into PSUM acc on PE; copy out on scalar
                nc.tensor.matmul(acc[:], identb[:], prod[:, 0, :, :],
                                 start=False, stop=True)
                res = resp.tile([128, 64, 8], BF16, tag="res",
                                name=f"res{q}{oct_}")
                nc.scalar.copy(res[:], acc[:])
                nc.sync.dma_start(out_d[q, oct_], res[:])

            prod_block(0, 0)
            prod_block(1, 0)
            kw_norm(1)
            prod_block(0, 1)
            prod_block(1, 1)

    nc.compile()
    ctx.__exit__(None, None, None)
    return nc


# ----------------------------------------------------------------------------
# host side
# ----------------------------------------------------------------------------
def _prep_weights(down_w, down_b, enc_w, enc_b, out_w, out_b):
    A = np.zeros((65, 65), np.float32)
    A[0:64, 0:64] = down_w[:, :, 0, 0]
    A[0:64, 64] = down_b
    A[64, 64] = 1.0
    ctap = np.zeros((65, 9, 25), np.float32)
    for dy in range(3):
        for dx in range(3):
            B = np.zeros((25, 65), np.float32)
            B[:, 0:64] = enc_w[:, :, dy, dx]
            if dy == 1 and dx == 1:
                B[:, 64] = enc_b
            ctap[:, 3 * dy + dx, :] = (B @ A).T
    w4 = out_w[:, :, 0, 0].T.reshape(2, 128, 64).astype(ml_dtypes.bfloat16)
    obf = np.broadcast_to(out_b[None, :, None], (128, 64, 8)).astype(
        ml_dtypes.bfloat16)
    return ctap.astype(ml_dtypes.bfloat16), w4, obf


def _slice_core(x, n, s):
    xk = np.zeros((65, 65, 258), np.float32)
    h0 = 64 * s - 1
    lo, hi = max(0, -h0), min(65, 256 - h0)
    xk[0:64, lo:hi, 1:257] = x[n, :, h0 + lo:h0 + hi, :]
    xk[64, lo:hi, 1:257] = 1.0
    xb = np.zeros((2, 128, 20, 264), np.float32)
    xbv = xb.reshape(256, 20, 264)
    for t in range(4):
        g0 = 64 * t + 16 * s - 2
        lo, hi = max(0, -g0), min(20, 256 - g0)
        xbv[np.arange(64) * 4 + t, lo:hi, 2:258] = x[n, :, g0 + lo:g0 + hi, :]
    return xk.astype(ml_dtypes.bfloat16), xb.astype(ml_dtypes.bfloat16)


_NC_CACHE = None
LAST_EXEC_NS = None


def kernel(x, down_w, down_b, enc_w, enc_b, out_w, out_b):
    global _NC_CACHE, LAST_EXEC_NS
    x = np.asarray(x, np.float32)
    ctap, w4, obf = _prep_weights(
        np.asarray(down_w, np.float32), np.asarray(down_b, np.float32),
        np.asarray(enc_w, np.float32), np.asarray(enc_b, np.float32),
        np.asarray(out_w, np.float32), np.asarray(out_b, np.float32))
    in_maps = []
    for core in range(N_CORES):
        n, s = core // 4, core % 4
        xk, xb = _slice_core(x, n, s)
        in_maps.append({"xk": xk, "xb": xb, "ctap": ctap, "w4": w4, "obf": obf})
    if _NC_CACHE is None:
        _NC_CACHE = build_nc()
    kw = {}
    if os.environ.get("CARAFE_TRACE"):
        kw = dict(trace=True, tmpdir=os.environ.get("CARAFE_TRACE_DIR"))
    res = run_bass_kernel_spmd(_NC_CACHE, in_maps, list(range(N_CORES)), **kw)
    if res.exec_time_ns is not None:
        LAST_EXEC_NS = res.exec_time_ns
    out = np.zeros((2, 64, 128, 128), np.float32)
    for core in range(N_CORES):
        n, s = core // 4, core % 4
        o = res.results[core]["out"].astype(np.float32)  # (q, oct, w', co, hh)
        # h' = 16*oct + 2*hh + q  ->  (co, h', w')
        o = o.transpose(3, 1, 4, 0, 2).reshape(64, 32, 128)
        out[n, :, 32 * s:32 * s + 32, :] = o
    return out
